# revision 7
# baseline (speedup 1.0000x reference)
"""ConvSwiGLU Trainium2 kernel: tensor-parallel over d_ff across 8 NeuronCores.

v3 design (from v2 profile: span 408us; ACT 89.8% busy (341us ACTIVATE: 26
ops/chunk at ~293ns fixed overhead each), DVE 93.2% (330us work + 49us
semaphore stalls), while PE only needs ~328us (786k cycles @2.4GHz) and was
never the real bottleneck):

  - All matmuls bf16 (fp8 DoubleRow is 2x but needs 3-term error
    compensation at this tolerance -> 1.5x bf16 cost; dead end).
  - mm1 bias, conv tap premultiply, and conv biases all fold into the tap
    ops: p_j = w_j*(mm1 + b) = (mm1 * w_j) + (w_j*b), via dual-scalar
    tensor_scalar on DVE (4x for even taps: 4B-aligned bf16 reads) and
    scale+bias activation on ACT (1x but shift-insensitive -> odd taps).
    cgb rides gate p2's additive const, cub rides up p2's. Host precomputes
    tap scale/bias tables [128, 2, K, GRP].
  - PSUM evacuations are pure wide copies: [128,2,512] 2-bank psum tiles
    evacuated in ONE ACTIVATE each (halves ACT op count; ACT costs
    (N+352)/1.2 ns so width is everything). Same for the down matmul.
  - One shared h mega-slab [128, 2(side), GRP, 2054] (halo zeros memset
    once); conv at chunk-pair granularity (width 1024): premults write
    [128, GRP, 1028] tiles, the add tree runs group-merged [128, GRP, 1024]
    tensor_tensors at 2x with even offsets only, Silu is group-merged, the
    swiglu mult emits per chunk so down(c) never waits on the full pair.
  - Per-pair engine budget: PE 40.8us, DVE ~29us (evens 4.6 + up-odds 4.8
    + adds 17.5 + mult 2.3), ACT ~31.6us (evacs 18.4 + gate-odds 9.2 +
    silu 4.0) -> PE-bound.
  - Down matmul: psum[m,t] = sum_f Wd[f,m] hact[f,t]; per-core partial yT
    summed on the host (bf16 partials, f32 host accumulate).
"""

import os
import sys
from contextlib import ExitStack

import ml_dtypes
import numpy as np

for _p in ("/root/.axon_site/_ro/trn_rl_repo", "/opt/trn_rl_repo"):
    if os.path.isdir(_p) and _p not in sys.path:
        sys.path.append(_p)

import concourse.bass as bass
import concourse.tile as tile
from concourse import bacc, mybir
from concourse.bass_utils import run_bass_kernel_spmd

F32 = mybir.dt.float32
BF16 = mybir.dt.bfloat16
AF = mybir.ActivationFunctionType
ALU = mybir.AluOpType

B, L, D = 4, 2048, 1024
F = 4096
NCORES = 8
FS = F // NCORES          # 512 d_ff channels per core
KSUB = D // 128           # 8 contraction subtiles for gate/up
GRP = FS // 128           # 4 channel groups per core
MSUB = D // 128           # 8 output row subtiles for down matmul
T = 512                   # token chunk (psum bank limit for f32)
CPS = L // T              # 4 chunks per sequence
NCH = (B * L) // T        # 16 chunks
NSEQ = B                  # 4 sequences
K = 5                     # conv taps
SLAB_W = L + 6            # 2 halo + 2048 tokens + 2 halo + 2 pad
PW = 1024                 # conv pair width (2 chunks)
PRW = PW + 4              # premult tile width

_cache = {}


def _build_program():
    nc = bacc.Bacc("TRN2", target_bir_lowering=False, debug=False,
                   enable_asserts=False, num_devices=NCORES)

    xTc = nc.dram_tensor("xTc", [NCH, 128, KSUB, T], BF16, kind="ExternalInput").ap()
    wg = nc.dram_tensor("wgS", [128, KSUB, FS], BF16, kind="ExternalInput").ap()
    wu = nc.dram_tensor("wuS", [128, KSUB, FS], BF16, kind="ExternalInput").ap()
    wd = nc.dram_tensor("wdS", [128, GRP, D], BF16, kind="ExternalInput").ap()
    tsc = nc.dram_tensor("tscS", [128, 2, K, GRP], F32, kind="ExternalInput").ap()
    tbi = nc.dram_tensor("tbiS", [128, 2, K, GRP], F32, kind="ExternalInput").ap()
    yT = nc.dram_tensor("yT", [D, B * L], BF16, kind="ExternalOutput").ap()

    with tile.TileContext(nc) as tc, ExitStack() as ctx:
        consts = ctx.enter_context(tc.tile_pool(name="consts", bufs=1))
        xpool = ctx.enter_context(tc.tile_pool(name="x", bufs=2))
        pp1 = ctx.enter_context(tc.tile_pool(name="pp1", bufs=1))
        ab1 = ctx.enter_context(tc.tile_pool(name="ab1", bufs=1))
        ab2 = ctx.enter_context(tc.tile_pool(name="ab2", bufs=2))
        hapool = ctx.enter_context(tc.tile_pool(name="ha", bufs=3))
        outpool = ctx.enter_context(tc.tile_pool(name="out", bufs=3))
        ps_main = ctx.enter_context(tc.tile_pool(name="psm", bufs=2, space="PSUM"))
        ps_dn = ctx.enter_context(tc.tile_pool(name="psd", bufs=2, space="PSUM"))

        # resident weights / constants
        wg_sb = consts.tile([128, KSUB, FS], BF16)
        wu_sb = consts.tile([128, KSUB, FS], BF16)
        wd_sb = consts.tile([128, GRP, D], BF16)
        tsc_sb = consts.tile([128, 2, K, GRP], F32)
        tbi_sb = consts.tile([128, 2, K, GRP], F32)
        # const loads on the Activation DMA queue (x/out use the SP queue);
        # wg + smalls first so chunk 0 matmuls can start as early as possible
        nc.scalar.dma_start(wg_sb[:, 0:2], wg[:, 0:2])
        nc.scalar.dma_start(wg_sb[:, 2:8], wg[:, 2:8])
        nc.scalar.dma_start(tsc_sb[:], tsc)
        nc.scalar.dma_start(tbi_sb[:], tbi)
        nc.scalar.dma_start(wu_sb[:], wu)
        nc.scalar.dma_start(wd_sb[:], wd)

        # one shared h slab (sides x groups x padded seq); halo zeros written
        # once -- evacuations only ever touch the interior [2, 2+L)
        slab = consts.tile([128, 2, GRP, SLAB_W], BF16)
        nc.gpsimd.memset(slab[:, :, :, 0:2], 0.0)
        nc.gpsimd.memset(slab[:, :, :, 2 + L:SLAB_W], 0.0)

        ha_tiles = {}

        def produce(i):
            """gate/up matmuls for chunk i -> biasless h slab columns."""
            c = i % CPS
            xt = xpool.tile([128, KSUB, T], BF16, tag="xt")
            nc.sync.dma_start(xt[:], xTc[i])
            for sd, w_sb in ((0, wg_sb), (1, wu_sb)):
                for gp in range(2):
                    ps = ps_main.tile([128, 2, T], F32, tag="mm1")
                    for half in range(2):
                        g = gp * 2 + half
                        for ks in range(KSUB):
                            nc.tensor.matmul(
                                ps[:, half, :],
                                w_sb[:, ks, g * 128:(g + 1) * 128],
                                xt[:, ks, :],
                                start=(ks == 0), stop=(ks == KSUB - 1))
                    nc.scalar.activation(
                        slab[:, sd, 2 * gp:2 * gp + 2, 2 + c * T:2 + (c + 1) * T],
                        ps[:], AF.Identity)

        def conv_pair(s, h):
            """conv + swiglu for seq-local token pair window [h*1024, +1024)."""
            t0 = h * PW  # slab col of first needed halo token (token t0-2)
            P = {}
            rs = {}
            # premults for both sides first so ACT's odd taps overlap DVE
            for sd in (0, 1):
                for j in range(K):
                    pj = pp1.tile([128, GRP, PRW], BF16, tag=f"p{sd}_{j}")
                    P[sd, j] = pj
                    dlt = j % 2
                    off = t0 + dlt
                    for g in range(GRP):
                        src = slab[:, sd, g, off:off + PRW]
                        sc = tsc_sb[:, sd, j, g:g + 1]
                        bi = tbi_sb[:, sd, j, g:g + 1]
                        if dlt and sd == 0:
                            # gate odd taps on ACT: 1x but shift-insensitive
                            nc.scalar.activation(pj[:, g, :], src, AF.Identity,
                                                 bias=bi, scale=sc)
                        else:
                            # DVE: even taps 4x (4B-aligned), up odds 2x_2p
                            nc.vector.tensor_scalar(pj[:, g, :], src, sc, bi,
                                                    ALU.mult, ALU.add)
            # group-merged add trees (DVE tensor_tensor 2x, even offsets);
            # tc aliases into p0's slot and r into p4's slot (both consumed
            # by ta already) to stay inside the SBUF budget
            for sd in (0, 1):
                p0, p1, p2, p3, p4 = (P[sd, j] for j in range(K))
                ta = ab1.tile([128, GRP, PW], BF16, tag="ta")
                nc.vector.tensor_tensor(ta[:], p0[:, :, 0:PW],
                                        p4[:, :, 4:4 + PW], ALU.add)
                tb = ab1.tile([128, GRP, PW], BF16, tag="tb")
                nc.vector.tensor_tensor(tb[:], p1[:, :, 0:PW],
                                        p3[:, :, 2:2 + PW], ALU.add)
                tc_ = p0[:, :, 0:PW]
                nc.vector.tensor_tensor(tc_, ta[:], tb[:], ALU.add)
                nc.vector.tensor_tensor(p4[:, :, 0:PW], tc_,
                                        p2[:, :, 2:2 + PW], ALU.add)
                rs[sd] = p4
            # silu + swiglu combine per chunk so down(c) starts early
            for q in (0, 1):
                off = q * T
                gact = ab2.tile([128, GRP, T], BF16, tag="gact")
                nc.scalar.activation(gact[:], rs[0][:, :, off:off + T], AF.Silu)
                ha_t = hapool.tile([128, GRP, T], BF16, tag="ha")
                nc.vector.tensor_tensor(ha_t[:], gact[:],
                                        rs[1][:, :, off:off + T], ALU.mult)
                ha_tiles[4 * s + 2 * h + q] = ha_t

        yTr = yT.rearrange("(ms p) t -> p ms t", p=128)

        def down(i):
            """down matmul + output DMA for chunk i."""
            ha_t = ha_tiles.pop(i)
            for mp in range(4):
                dps = ps_dn.tile([128, 2, T], F32, tag="dn")
                for half in range(2):
                    ms = mp * 2 + half
                    for g in range(GRP):
                        nc.tensor.matmul(
                            dps[:, half, :],
                            wd_sb[:, g, ms * 128:(ms + 1) * 128],
                            ha_t[:, g, :],
                            start=(g == 0), stop=(g == GRP - 1))
                ot = outpool.tile([128, 2, T], BF16, tag="out")
                nc.scalar.activation(ot[:], dps[:], AF.Identity)
                nc.gpsimd.dma_start(
                    yTr[:, 2 * mp:2 * mp + 2, i * T:(i + 1) * T], ot[:])

        for s in range(NSEQ):
            produce(4 * s)
            produce(4 * s + 1)
            produce(4 * s + 2)
            conv_pair(s, 0)
            produce(4 * s + 3)
            down(4 * s)
            conv_pair(s, 1)
            down(4 * s + 1)
            down(4 * s + 2)
            down(4 * s + 3)

    nc.compile()
    return nc


def _prep_inputs(x, Wg, bgv, Wu, buv, convg_w, convg_b, convu_w, convu_b, Wd):
    """Host-side shard/layout. Returns list of per-core in_maps."""
    bf16 = ml_dtypes.bfloat16
    x = np.ascontiguousarray(x, np.float32)
    # [B, L, D] -> [B, KSUB, 128, L] -> chunks [NCH, 128, KSUB, T]
    xt = x.transpose(0, 2, 1).reshape(B, KSUB, 128, L)
    xTc = np.stack([
        xt[i // CPS, :, :, (i % CPS) * T:(i % CPS + 1) * T].transpose(1, 0, 2)
        for i in range(NCH)
    ]).astype(bf16)

    def colsplit(w, c):      # [D, F] -> per-core [128, KSUB, FS]
        s = np.asarray(w, np.float32)[:, c * FS:(c + 1) * FS]
        return np.ascontiguousarray(
            s.reshape(KSUB, 128, FS).transpose(1, 0, 2)).astype(bf16)

    def vecsplit(v, c):      # [F] -> [128, GRP] f32
        return np.ascontiguousarray(
            np.asarray(v, np.float32)[c * FS:(c + 1) * FS].reshape(GRP, 128).T)

    def tapsplit(w, c):      # [F, 1, K] -> [128, GRP, K] f32
        return np.ascontiguousarray(
            np.asarray(w, np.float32)[c * FS:(c + 1) * FS, 0, :]
            .reshape(GRP, 128, K).transpose(1, 0, 2))

    in_maps = []
    for c in range(NCORES):
        wdS = np.asarray(Wd, np.float32)[c * FS:(c + 1) * FS, :]
        # tap scale/bias tables [128, 2, K, GRP]:
        #   p_j = (mm1 * w_j) + (w_j * b_side); conv biases ride tap j=2
        tscS = np.zeros((128, 2, K, GRP), np.float32)
        tbiS = np.zeros((128, 2, K, GRP), np.float32)
        for sd, (cw, bv, cb) in enumerate(((convg_w, bgv, convg_b),
                                           (convu_w, buv, convu_b))):
            taps = tapsplit(cw, c)            # [128, GRP, K]
            bias = vecsplit(bv, c)            # [128, GRP]
            tscS[:, sd] = taps.transpose(0, 2, 1)
            tbiS[:, sd] = (taps * bias[:, :, None]).transpose(0, 2, 1)
            tbiS[:, sd, 2] += vecsplit(cb, c)
        in_maps.append({
            "xTc": xTc,
            "wgS": colsplit(Wg, c),
            "wuS": colsplit(Wu, c),
            "wdS": np.ascontiguousarray(
                wdS.reshape(GRP, 128, D).transpose(1, 0, 2)).astype(bf16),
            "tscS": tscS,
            "tbiS": tbiS,
        })
    return in_maps


def run_on_cores(in_maps, **kwargs):
    if "nc" not in _cache:
        _cache["nc"] = _build_program()
    return run_bass_kernel_spmd(_cache["nc"], in_maps,
                                core_ids=list(range(NCORES)), **kwargs)


def kernel(x, Wg, bg, Wu, bu, convg_w, convg_b, convu_w, convu_b, Wd, bd):
    in_maps = _prep_inputs(x, Wg, bg, Wu, bu, convg_w, convg_b,
                           convu_w, convu_b, Wd)
    res = run_on_cores(in_maps)
    acc = np.zeros((D, B * L), np.float32)
    for r in res.results:
        acc += np.asarray(r["yT"], np.float32)
    acc += np.asarray(bd, np.float32)[:, None]
    return np.ascontiguousarray(acc.T.reshape(B, L, D)).astype(np.float32)


# revision 11
# speedup vs baseline: 1.2801x; 1.2801x over previous
"""ConvSwiGLU Trainium2 kernel: tensor-parallel over d_ff across 8 NeuronCores.

v3.1 design (v2 profile: span 408us; ACT 89.8% busy (341us ACTIVATE: 26
ops/chunk at ~293ns fixed overhead each), DVE 93.2% (330us work + 49us
semaphore stalls), while PE only needs ~328us (786k cycles @2.4GHz) and was
never the real bottleneck. v3 lessons: dual-scalar tensor_scalar loses the
fast DVE modes (661ns vs 395 predicted) and group-merged 3D tensor_tensor
pays ~150-220 cyc per extra row, so DVE must run single-scalar single-row;
merges only pay on ACT where they amortize its 352-cycle op overhead):

  - All matmuls bf16 (fp8 DoubleRow is 2x but needs 3-term error
    compensation at this tolerance -> 1.5x bf16 cost; dead end).
  - PSUM evacuations are pure wide copies: [128,2,512] 2-bank psum tiles
    evacuated in ONE ACTIVATE each (halves ACT op count; ACT costs about
    (N+352)/1.2 ns + ~220cyc/extra row). Same for the down matmul.
  - Conv premults are pure single-scalar tensor_scalar mults: j0/j4 at 4x
    (4B-aligned bf16), j3 + up j1 at 2x_2p; ACT takes the center taps and
    gate j1 (1x, shift-insensitive). The whole mm1 bias rides the center
    tap's bias slot as b*sum(w_j) + conv_b, and the slab halo columns hold
    -b (DMA'd per-partition constants) so pad taps contribute exactly 0.
  - One shared h mega-slab [128, 2(side), GRP, 2054]; conv at chunk-pair
    granularity (width 1024): premult tiles [128, GRP, 1028], add tree as
    per-group single-row [1024] tensor_tensors at 2x with even offsets
    (tc aliases into p0, r into p4 to fit SBUF), Silu group-merged with
    bias=0, swiglu mult per group.
  - Downs of pair 1 spread into the next sequence's produce slots so PE
    alternates produce/down chunks and ACT sees a regular evac cadence.
  - Per-pair engine budget: PE 40.8us, DVE ~34us, ACT ~31us -> PE-bound.
  - Down matmul: psum[m,t] = sum_f Wd[f,m] hact[f,t]; per-core partial yT
    summed on the host (bf16 partials, f32 host accumulate).
"""

import os
import sys
from contextlib import ExitStack

import ml_dtypes
import numpy as np

for _p in ("/root/.axon_site/_ro/trn_rl_repo", "/opt/trn_rl_repo"):
    if os.path.isdir(_p) and _p not in sys.path:
        sys.path.append(_p)

import concourse.bass as bass
import concourse.tile as tile
from concourse import bacc, mybir
from concourse.bass_utils import run_bass_kernel_spmd

F32 = mybir.dt.float32
BF16 = mybir.dt.bfloat16
AF = mybir.ActivationFunctionType
ALU = mybir.AluOpType

B, L, D = 4, 2048, 1024
F = 4096
NCORES = 8
FS = F // NCORES          # 512 d_ff channels per core
KSUB = D // 128           # 8 contraction subtiles for gate/up
GRP = FS // 128           # 4 channel groups per core
MSUB = D // 128           # 8 output row subtiles for down matmul
T = 512                   # token chunk (psum bank limit for f32)
CPS = L // T              # 4 chunks per sequence
NCH = (B * L) // T        # 16 chunks
NSEQ = B                  # 4 sequences
K = 5                     # conv taps
SLAB_W = L + 6            # 2 halo + 2048 tokens + 4 halo/pad
PW = 1024                 # conv pair width (2 chunks)
PRW = PW + 4              # premult tile width

_cache = {}


def _build_program():
    nc = bacc.Bacc("TRN2", target_bir_lowering=False, debug=False,
                   enable_asserts=False, num_devices=NCORES)

    xTc = nc.dram_tensor("xTc", [NCH, 128, KSUB, T], BF16, kind="ExternalInput").ap()
    wg = nc.dram_tensor("wgS", [128, KSUB, FS], BF16, kind="ExternalInput").ap()
    wu = nc.dram_tensor("wuS", [128, KSUB, FS], BF16, kind="ExternalInput").ap()
    wd = nc.dram_tensor("wdS", [128, GRP, D], BF16, kind="ExternalInput").ap()
    tsc = nc.dram_tensor("tscS", [128, 2, K, GRP], F32, kind="ExternalInput").ap()
    tb2 = nc.dram_tensor("tb2S", [128, 2, GRP], F32, kind="ExternalInput").ap()
    nbh = nc.dram_tensor("nbhS", [128, 2, GRP, 6], BF16, kind="ExternalInput").ap()
    yT = nc.dram_tensor("yT", [D, B * L], BF16, kind="ExternalOutput").ap()

    with tile.TileContext(nc) as tc, ExitStack() as ctx:
        consts = ctx.enter_context(tc.tile_pool(name="consts", bufs=1))
        xpool = ctx.enter_context(tc.tile_pool(name="x", bufs=2))
        pp1 = ctx.enter_context(tc.tile_pool(name="pp1", bufs=1))
        ab1 = ctx.enter_context(tc.tile_pool(name="ab1", bufs=1))
        ab2 = ctx.enter_context(tc.tile_pool(name="ab2", bufs=2))
        hapool = ctx.enter_context(tc.tile_pool(name="ha", bufs=2))
        outpool = ctx.enter_context(tc.tile_pool(name="out", bufs=2))
        ps_main = ctx.enter_context(tc.tile_pool(name="psm", bufs=2, space="PSUM"))
        ps_dn = ctx.enter_context(tc.tile_pool(name="psd", bufs=2, space="PSUM"))

        # resident weights / constants
        wg_sb = consts.tile([128, KSUB, FS], BF16)
        wu_sb = consts.tile([128, KSUB, FS], BF16)
        wd_sb = consts.tile([128, GRP, D], BF16)
        tsc_sb = consts.tile([128, 2, K, GRP], F32)
        tb2_sb = consts.tile([128, 2, GRP], F32)
        # const loads on the Activation DMA queue (x/out use the SP queue);
        # wg + smalls first so chunk 0 matmuls can start as early as possible
        nc.scalar.dma_start(wg_sb[:, 0:2], wg[:, 0:2])
        nc.scalar.dma_start(wg_sb[:, 2:8], wg[:, 2:8])
        nc.scalar.dma_start(tsc_sb[:], tsc)
        nc.scalar.dma_start(tb2_sb[:], tb2)
        nc.scalar.dma_start(wu_sb[:], wu)
        nc.scalar.dma_start(wd_sb[:], wd)

        # one shared h slab (sides x groups x padded seq); halo columns hold
        # -b so pure-mult premults contribute exactly 0 at pad positions
        # (the full bias b*sum_j(w_j) + conv_b rides the center tap).
        # Evacuations only ever touch the interior [2, 2+L).
        slab = consts.tile([128, 2, GRP, SLAB_W], BF16)
        nc.gpsimd.dma_start(slab[:, :, :, 0:2], nbh[:, :, :, 0:2])
        nc.gpsimd.dma_start(slab[:, :, :, 2 + L:SLAB_W], nbh[:, :, :, 2:6])

        ha_tiles = {}

        def produce(i):
            """gate/up matmuls for chunk i -> biasless h slab columns."""
            c = i % CPS
            xt = xpool.tile([128, KSUB, T], BF16, tag="xt")
            nc.sync.dma_start(xt[:], xTc[i])
            for sd, w_sb in ((0, wg_sb), (1, wu_sb)):
                for gp in range(2):
                    ps = ps_main.tile([128, 2, T], F32, tag="mm1")
                    for half in range(2):
                        g = gp * 2 + half
                        for ks in range(KSUB):
                            nc.tensor.matmul(
                                ps[:, half, :],
                                w_sb[:, ks, g * 128:(g + 1) * 128],
                                xt[:, ks, :],
                                start=(ks == 0), stop=(ks == KSUB - 1))
                    nc.scalar.activation(
                        slab[:, sd, 2 * gp:2 * gp + 2, 2 + c * T:2 + (c + 1) * T],
                        ps[:], AF.Identity)

        def conv_pair(s, h):
            """conv + swiglu for seq-local token pair window [h*1024, +1024)."""
            t0 = h * PW
            P = {}
            # premults first so ACT's taps overlap DVE's
            for sd in (0, 1):
                for j in range(K):
                    pj = pp1.tile([128, GRP, PRW], BF16, tag=f"p{sd}_{j}")
                    P[sd, j] = pj
                    dlt = j % 2
                    for g in range(GRP):
                        src = slab[:, sd, g, t0 + dlt:t0 + dlt + PRW]
                        sc = tsc_sb[:, sd, j, g:g + 1]
                        if j == 2 or (j == 1 and sd == 0):
                            # ACT: 1x, shift-insensitive; center tap carries
                            # the whole folded bias
                            bias = tb2_sb[:, sd, g:g + 1] if j == 2 else 0.0
                            nc.scalar.activation(pj[:, g, :], src, AF.Identity,
                                                 bias=bias, scale=sc)
                        else:
                            # DVE: j0/j4 at 4x (aligned), j3 + up j1 at 2x_2p
                            nc.vector.tensor_scalar(pj[:, g, :], src, sc, None,
                                                    ALU.mult)
            # per-group single-row add trees (DVE tensor_tensor 2x, even
            # offsets); tc aliases into p0's row and r into p4's row (both
            # already consumed by ta) to stay inside the SBUF budget
            for sd in (0, 1):
                p0, p1, p2, p3, p4 = (P[sd, j] for j in range(K))
                for g in range(GRP):
                    ta = ab1.tile([128, PW], BF16, tag="ta")
                    tb = ab1.tile([128, PW], BF16, tag="tb")
                    nc.vector.tensor_tensor(ta[:], p0[:, g, 0:PW],
                                            p4[:, g, 4:4 + PW], ALU.add)
                    nc.vector.tensor_tensor(tb[:], p1[:, g, 0:PW],
                                            p3[:, g, 2:2 + PW], ALU.add)
                    nc.vector.tensor_tensor(p0[:, g, 0:PW], ta[:], tb[:],
                                            ALU.add)
                    nc.vector.tensor_tensor(p4[:, g, 0:PW], p0[:, g, 0:PW],
                                            p2[:, g, 2:2 + PW], ALU.add)
            # per-chunk group-merged silu (bias already folded) + swiglu
            ha_t = hapool.tile([128, GRP, PW], BF16, tag="ha")
            for q in (0, 1):
                off = q * T
                gact = ab2.tile([128, GRP, T], BF16, tag="gact")
                nc.scalar.activation(gact[:], P[0, 4][:, :, off:off + T],
                                     AF.Silu)
                for g in range(GRP):
                    nc.vector.tensor_tensor(ha_t[:, g, off:off + T],
                                            gact[:, g, :],
                                            P[1, 4][:, g, off:off + T],
                                            ALU.mult)
            ha_tiles[4 * s + 2 * h] = ha_t

        yTr = yT.rearrange("(ms p) t -> p ms t", p=128)

        def down(i):
            """down matmul + output DMA for chunk i."""
            pair = i // 2
            ha_t = ha_tiles[2 * pair]
            off = (i % 2) * T
            out_sb = outpool.tile([128, MSUB, T], BF16, tag="out")
            for mp in range(4):
                dps = ps_dn.tile([128, 2, T], F32, tag="dn")
                for half in range(2):
                    ms = mp * 2 + half
                    for g in range(GRP):
                        nc.tensor.matmul(
                            dps[:, half, :],
                            wd_sb[:, g, ms * 128:(ms + 1) * 128],
                            ha_t[:, g, off:off + T],
                            start=(g == 0), stop=(g == GRP - 1))
                nc.scalar.activation(out_sb[:, 2 * mp:2 * mp + 2, :], dps[:],
                                     AF.Identity)
            nc.gpsimd.dma_start(yTr[:, :, i * T:(i + 1) * T], out_sb[:])

        # software pipeline: downs of pair 1 spread into the next sequence's
        # produce slots so PE alternates produce/down chunks
        for s in range(NSEQ):
            produce(4 * s)
            if s > 0:
                down(4 * s - 2)
            produce(4 * s + 1)
            if s > 0:
                down(4 * s - 1)
            produce(4 * s + 2)
            conv_pair(s, 0)
            produce(4 * s + 3)
            down(4 * s)
            conv_pair(s, 1)
            down(4 * s + 1)
        down(NCH - 2)
        down(NCH - 1)

    nc.compile()
    return nc


def _prep_inputs(x, Wg, bgv, Wu, buv, convg_w, convg_b, convu_w, convu_b, Wd):
    """Host-side shard/layout. Returns list of per-core in_maps."""
    bf16 = ml_dtypes.bfloat16
    x = np.ascontiguousarray(x, np.float32)
    # [B, L, D] -> [B, KSUB, 128, L] -> chunks [NCH, 128, KSUB, T]
    xt = x.transpose(0, 2, 1).reshape(B, KSUB, 128, L)
    xTc = np.stack([
        xt[i // CPS, :, :, (i % CPS) * T:(i % CPS + 1) * T].transpose(1, 0, 2)
        for i in range(NCH)
    ]).astype(bf16)

    def colsplit(w, c):      # [D, F] -> per-core [128, KSUB, FS]
        s = np.asarray(w, np.float32)[:, c * FS:(c + 1) * FS]
        return np.ascontiguousarray(
            s.reshape(KSUB, 128, FS).transpose(1, 0, 2)).astype(bf16)

    def vecsplit(v, c):      # [F] -> [128, GRP] f32
        return np.ascontiguousarray(
            np.asarray(v, np.float32)[c * FS:(c + 1) * FS].reshape(GRP, 128).T)

    def tapsplit(w, c):      # [F, 1, K] -> [128, GRP, K] f32
        return np.ascontiguousarray(
            np.asarray(w, np.float32)[c * FS:(c + 1) * FS, 0, :]
            .reshape(GRP, 128, K).transpose(1, 0, 2))

    in_maps = []
    for c in range(NCORES):
        wdS = np.asarray(Wd, np.float32)[c * FS:(c + 1) * FS, :]
        # tap scales [128, 2, K, GRP]; folded center bias b*sum(w_j)+conv_b
        # [128, 2, GRP]; -b halo fill [128, 2, GRP, 6]
        tscS = np.zeros((128, 2, K, GRP), np.float32)
        tb2S = np.zeros((128, 2, GRP), np.float32)
        nbhS = np.zeros((128, 2, GRP, 6), np.float32)
        for sd, (cw, bv, cb) in enumerate(((convg_w, bgv, convg_b),
                                           (convu_w, buv, convu_b))):
            taps = tapsplit(cw, c)            # [128, GRP, K]
            bias = vecsplit(bv, c)            # [128, GRP]
            tscS[:, sd] = taps.transpose(0, 2, 1)
            tb2S[:, sd] = bias * taps.sum(axis=2) + vecsplit(cb, c)
            nbhS[:, sd] = -bias[:, :, None]
        in_maps.append({
            "xTc": xTc,
            "wgS": colsplit(Wg, c),
            "wuS": colsplit(Wu, c),
            "wdS": np.ascontiguousarray(
                wdS.reshape(GRP, 128, D).transpose(1, 0, 2)).astype(bf16),
            "tscS": tscS,
            "tb2S": tb2S,
            "nbhS": nbhS.astype(bf16),
        })
    return in_maps


def run_on_cores(in_maps, **kwargs):
    if "nc" not in _cache:
        _cache["nc"] = _build_program()
    return run_bass_kernel_spmd(_cache["nc"], in_maps,
                                core_ids=list(range(NCORES)), **kwargs)


def kernel(x, Wg, bg, Wu, bu, convg_w, convg_b, convu_w, convu_b, Wd, bd):
    in_maps = _prep_inputs(x, Wg, bg, Wu, bu, convg_w, convg_b,
                           convu_w, convu_b, Wd)
    res = run_on_cores(in_maps)
    acc = np.zeros((D, B * L), np.float32)
    for r in res.results:
        acc += np.asarray(r["yT"], np.float32)
    acc += np.asarray(bd, np.float32)[:, None]
    return np.ascontiguousarray(acc.T.reshape(B, L, D)).astype(np.float32)


# revision 12
# speedup vs baseline: 1.2805x; 1.0003x over previous
"""ConvSwiGLU Trainium2 kernel: tensor-parallel over d_ff across 8 NeuronCores.

v3.2 design. Profile history: v2 408us span (ACT 90%/DVE 93% busy, PE only
needs ~348us incl. chain bubbles); v3 537us (dual-scalar tensor_scalar loses
the fast DVE modes; group-merged 3D tensor_tensor pays ~150-220cyc/extra
row); v3.1 426us (DVE 76%, ACT 82%, but PE idled 78us: 17us head waiting on
the full 1MB x chunk DMA, 15us tail, 43us of mid-gaps where mm1 evacs
queued behind 15-19us conv-tap batches on ACT with only 2 psum tiles of
slack).

  - All matmuls bf16 (fp8 DoubleRow is 2x but needs 3-term error
    compensation at this tolerance -> 1.5x bf16 cost; dead end).
  - PSUM evacuations are pure wide copies: [128,2,512] 2-bank psum tiles
    evacuated in ONE ACTIVATE each; ACT costs ~(N+352)/1.2 ns +220cyc/row.
  - Conv at CHUNK granularity (width 512, premult tiles [128,GRP,516]) in
    a strict produce(i) / conv(i-1) / down(i-2) cadence: per chunk ACT does
    4 mm1 evacs, 8 j3 taps, 1 merged silu, 4 down evacs (~16.7us) and DVE
    does 24 even taps @4x, 8 j1 taps @2x_2p, 16 adds, 4 mults (~15.7us),
    both under PE's 20.4us; ACT tap batches are small so evacs never
    backlog behind them (v3.1's 43us of PE mid-gaps).
  - Taps are pure single-scalar mults except j3 (ACT, shift-insensitive)
    which carries the whole folded bias b*sum(w_j)+conv_b; slab halo
    columns hold -b so pad taps contribute exactly 0 (bias-exact edges).
  - x chunk DMA split 4 ways so chunk 0's first matmuls start ~7us sooner
    (single 1MB DMA ran ~10us on one queue).
  - One shared h mega-slab [128, 2(side), GRP, 2054]; add tree per group
    at even offsets, tc aliasing into p0 and r into p4; silu group-merged
    (bias=0); swiglu mult per group into [128,GRP,512] ha tiles.
  - Down matmul: psum[m,t] = sum_f Wd[f,m] hact[f,t]; per-core partial yT
    summed on the host (bf16 partials, f32 host accumulate).
"""

import os
import sys
from contextlib import ExitStack

import ml_dtypes
import numpy as np

for _p in ("/root/.axon_site/_ro/trn_rl_repo", "/opt/trn_rl_repo"):
    if os.path.isdir(_p) and _p not in sys.path:
        sys.path.append(_p)

import concourse.bass as bass
import concourse.tile as tile
from concourse import bacc, mybir
from concourse.bass_utils import run_bass_kernel_spmd

F32 = mybir.dt.float32
BF16 = mybir.dt.bfloat16
AF = mybir.ActivationFunctionType
ALU = mybir.AluOpType

B, L, D = 4, 2048, 1024
F = 4096
NCORES = 8
FS = F // NCORES          # 512 d_ff channels per core
KSUB = D // 128           # 8 contraction subtiles for gate/up
GRP = FS // 128           # 4 channel groups per core
MSUB = D // 128           # 8 output row subtiles for down matmul
T = 512                   # token chunk (psum bank limit for f32)
CPS = L // T              # 4 chunks per sequence
NCH = (B * L) // T        # 16 chunks
NSEQ = B                  # 4 sequences
K = 5                     # conv taps
SLAB_W = L + 6            # 2 halo + 2048 tokens + 4 halo/pad
PRW = T + 4               # premult tile width

_cache = {}


def _build_program():
    nc = bacc.Bacc("TRN2", target_bir_lowering=False, debug=False,
                   enable_asserts=False, num_devices=NCORES)

    xTc = nc.dram_tensor("xTc", [NCH, 128, KSUB, T], BF16, kind="ExternalInput").ap()
    wg = nc.dram_tensor("wgS", [128, KSUB, FS], BF16, kind="ExternalInput").ap()
    wu = nc.dram_tensor("wuS", [128, KSUB, FS], BF16, kind="ExternalInput").ap()
    wd = nc.dram_tensor("wdS", [128, GRP, D], BF16, kind="ExternalInput").ap()
    tsc = nc.dram_tensor("tscS", [128, 2, K, GRP], F32, kind="ExternalInput").ap()
    tb2 = nc.dram_tensor("tb2S", [128, 2, GRP], F32, kind="ExternalInput").ap()
    nbh = nc.dram_tensor("nbhS", [128, 2, GRP, 6], BF16, kind="ExternalInput").ap()
    yT = nc.dram_tensor("yT", [D, B * L], BF16, kind="ExternalOutput").ap()

    with tile.TileContext(nc) as tc, ExitStack() as ctx:
        consts = ctx.enter_context(tc.tile_pool(name="consts", bufs=1))
        xpool = ctx.enter_context(tc.tile_pool(name="x", bufs=3))
        pp1 = ctx.enter_context(tc.tile_pool(name="pp1", bufs=1))
        pp2 = ctx.enter_context(tc.tile_pool(name="pp2", bufs=2))
        ab1 = ctx.enter_context(tc.tile_pool(name="ab1", bufs=2))
        ab2 = ctx.enter_context(tc.tile_pool(name="ab2", bufs=2))
        hapool = ctx.enter_context(tc.tile_pool(name="ha", bufs=3))
        outpool = ctx.enter_context(tc.tile_pool(name="out", bufs=3))
        ps_main = ctx.enter_context(tc.tile_pool(name="psm", bufs=2, space="PSUM"))
        ps_dn = ctx.enter_context(tc.tile_pool(name="psd", bufs=2, space="PSUM"))

        # resident weights / constants
        wg_sb = consts.tile([128, KSUB, FS], BF16)
        wu_sb = consts.tile([128, KSUB, FS], BF16)
        wd_sb = consts.tile([128, GRP, D], BF16)
        tsc_sb = consts.tile([128, 2, K, GRP], F32)
        tb2_sb = consts.tile([128, 2, GRP], F32)
        # const loads on the Activation DMA queue (x/out use the SP queue);
        # wg first 2 ksubs first so chunk 0 matmuls can start immediately
        nc.scalar.dma_start(wg_sb[:, 0:2], wg[:, 0:2])
        nc.scalar.dma_start(wg_sb[:, 2:8], wg[:, 2:8])
        nc.scalar.dma_start(wu_sb[:], wu)
        nc.scalar.dma_start(wd_sb[:], wd)
        nc.scalar.dma_start(tsc_sb[:], tsc)
        nc.scalar.dma_start(tb2_sb[:], tb2)

        # one shared h slab (sides x groups x padded seq); halo columns hold
        # -b so pure-mult premults contribute exactly 0 at pad positions
        # (the full bias b*sum_j(w_j) + conv_b rides the j3 tap).
        # Evacuations only ever touch the interior [2, 2+L).
        slab = consts.tile([128, 2, GRP, SLAB_W], BF16)
        nc.gpsimd.dma_start(slab[:, :, :, 0:2], nbh[:, :, :, 0:2])
        nc.gpsimd.dma_start(slab[:, :, :, 2 + L:SLAB_W], nbh[:, :, :, 2:6])

        ha_tiles = {}

        def produce(i):
            """gate/up matmuls for chunk i -> biasless h slab columns."""
            c = i % CPS
            xt = xpool.tile([128, KSUB, T], BF16, tag="xt")
            for k0 in range(0, KSUB, 2):
                nc.sync.dma_start(xt[:, k0:k0 + 2], xTc[i, :, k0:k0 + 2])
            for sd, w_sb in ((0, wg_sb), (1, wu_sb)):
                for gp in range(2):
                    ps = ps_main.tile([128, 2, T], F32, tag="mm1")
                    for half in range(2):
                        g = gp * 2 + half
                        for ks in range(KSUB):
                            nc.tensor.matmul(
                                ps[:, half, :],
                                w_sb[:, ks, g * 128:(g + 1) * 128],
                                xt[:, ks, :],
                                start=(ks == 0), stop=(ks == KSUB - 1))
                    nc.scalar.activation(
                        slab[:, sd, 2 * gp:2 * gp + 2, 2 + c * T:2 + (c + 1) * T],
                        ps[:], AF.Identity)

        def conv(i):
            """conv + swiglu for chunk i (needs first cols of chunk i+1)."""
            c = i % CPS
            t0 = c * T
            P = {}
            # premults first so ACT's j3 taps overlap DVE's
            for sd in (0, 1):
                for j in range(K):
                    pool = pp2 if j == 3 else pp1
                    pj = pool.tile([128, GRP, PRW], BF16, tag=f"p{sd}_{j}")
                    P[sd, j] = pj
                    dlt = j % 2
                    for g in range(GRP):
                        src = slab[:, sd, g, t0 + dlt:t0 + dlt + PRW]
                        sc = tsc_sb[:, sd, j, g:g + 1]
                        if j == 3:
                            # ACT: 1x, shift-insensitive, carries the whole
                            # folded bias b*sum(w_j) + conv_b
                            nc.scalar.activation(pj[:, g, :], src, AF.Identity,
                                                 bias=tb2_sb[:, sd, g:g + 1],
                                                 scale=sc)
                        else:
                            # DVE: j0/j2/j4 at 4x (aligned), j1 at 2x_2p
                            nc.vector.tensor_scalar(pj[:, g, :], src, sc, None,
                                                    ALU.mult)
            # per-group add trees (DVE tensor_tensor 2x, even offsets);
            # tc aliases into p0's row and r into p4's row (both already
            # consumed by ta)
            for sd in (0, 1):
                p0, p1, p2, p3, p4 = (P[sd, j] for j in range(K))
                for g in range(GRP):
                    ta = ab1.tile([128, T], BF16, tag="ta")
                    tb = ab1.tile([128, T], BF16, tag="tb")
                    nc.vector.tensor_tensor(ta[:], p0[:, g, 0:T],
                                            p4[:, g, 4:4 + T], ALU.add)
                    nc.vector.tensor_tensor(tb[:], p1[:, g, 0:T],
                                            p3[:, g, 2:2 + T], ALU.add)
                    nc.vector.tensor_tensor(p0[:, g, 0:T], ta[:], tb[:],
                                            ALU.add)
                    nc.vector.tensor_tensor(p4[:, g, 0:T], p0[:, g, 0:T],
                                            p2[:, g, 2:2 + T], ALU.add)
            # group-merged silu (bias already folded) + per-group swiglu
            gact = ab2.tile([128, GRP, T], BF16, tag="gact")
            nc.scalar.activation(gact[:], P[0, 4][:, :, 0:T], AF.Silu)
            ha_t = hapool.tile([128, GRP, T], BF16, tag="ha")
            for g in range(GRP):
                nc.vector.tensor_tensor(ha_t[:, g, :], gact[:, g, :],
                                        P[1, 4][:, g, 0:T], ALU.mult)
            ha_tiles[i] = ha_t

        yTr = yT.rearrange("(ms p) t -> p ms t", p=128)

        def down(i):
            """down matmul + output DMA for chunk i."""
            ha_t = ha_tiles.pop(i)
            out_sb = outpool.tile([128, MSUB, T], BF16, tag="out")
            for mp in range(4):
                dps = ps_dn.tile([128, 2, T], F32, tag="dn")
                for half in range(2):
                    ms = mp * 2 + half
                    for g in range(GRP):
                        nc.tensor.matmul(
                            dps[:, half, :],
                            wd_sb[:, g, ms * 128:(ms + 1) * 128],
                            ha_t[:, g, :],
                            start=(g == 0), stop=(g == GRP - 1))
                nc.scalar.activation(out_sb[:, 2 * mp:2 * mp + 2, :], dps[:],
                                     AF.Identity)
            nc.gpsimd.dma_start(yTr[:, :, i * T:(i + 1) * T], out_sb[:])

        # strict cadence: produce(i) / conv(i-1) / down(i-2)
        for i in range(NCH):
            produce(i)
            if i >= 1:
                conv(i - 1)
            if i >= 2:
                down(i - 2)
        conv(NCH - 1)
        down(NCH - 2)
        down(NCH - 1)

    nc.compile()
    return nc


def _prep_inputs(x, Wg, bgv, Wu, buv, convg_w, convg_b, convu_w, convu_b, Wd):
    """Host-side shard/layout. Returns list of per-core in_maps."""
    bf16 = ml_dtypes.bfloat16
    x = np.ascontiguousarray(x, np.float32)
    # [B, L, D] -> [B, KSUB, 128, L] -> chunks [NCH, 128, KSUB, T]
    xt = x.transpose(0, 2, 1).reshape(B, KSUB, 128, L)
    xTc = np.stack([
        xt[i // CPS, :, :, (i % CPS) * T:(i % CPS + 1) * T].transpose(1, 0, 2)
        for i in range(NCH)
    ]).astype(bf16)

    def colsplit(w, c):      # [D, F] -> per-core [128, KSUB, FS]
        s = np.asarray(w, np.float32)[:, c * FS:(c + 1) * FS]
        return np.ascontiguousarray(
            s.reshape(KSUB, 128, FS).transpose(1, 0, 2)).astype(bf16)

    def vecsplit(v, c):      # [F] -> [128, GRP] f32
        return np.ascontiguousarray(
            np.asarray(v, np.float32)[c * FS:(c + 1) * FS].reshape(GRP, 128).T)

    def tapsplit(w, c):      # [F, 1, K] -> [128, GRP, K] f32
        return np.ascontiguousarray(
            np.asarray(w, np.float32)[c * FS:(c + 1) * FS, 0, :]
            .reshape(GRP, 128, K).transpose(1, 0, 2))

    in_maps = []
    for c in range(NCORES):
        wdS = np.asarray(Wd, np.float32)[c * FS:(c + 1) * FS, :]
        # tap scales [128, 2, K, GRP]; folded j3 bias b*sum(w_j)+conv_b
        # [128, 2, GRP]; -b halo fill [128, 2, GRP, 6]
        tscS = np.zeros((128, 2, K, GRP), np.float32)
        tb2S = np.zeros((128, 2, GRP), np.float32)
        nbhS = np.zeros((128, 2, GRP, 6), np.float32)
        for sd, (cw, bv, cb) in enumerate(((convg_w, bgv, convg_b),
                                           (convu_w, buv, convu_b))):
            taps = tapsplit(cw, c)            # [128, GRP, K]
            bias = vecsplit(bv, c)            # [128, GRP]
            tscS[:, sd] = taps.transpose(0, 2, 1)
            tb2S[:, sd] = bias * taps.sum(axis=2) + vecsplit(cb, c)
            nbhS[:, sd] = -bias[:, :, None]
        in_maps.append({
            "xTc": xTc,
            "wgS": colsplit(Wg, c),
            "wuS": colsplit(Wu, c),
            "wdS": np.ascontiguousarray(
                wdS.reshape(GRP, 128, D).transpose(1, 0, 2)).astype(bf16),
            "tscS": tscS,
            "tb2S": tb2S,
            "nbhS": nbhS.astype(bf16),
        })
    return in_maps


def run_on_cores(in_maps, **kwargs):
    if "nc" not in _cache:
        _cache["nc"] = _build_program()
    return run_bass_kernel_spmd(_cache["nc"], in_maps,
                                core_ids=list(range(NCORES)), **kwargs)


def kernel(x, Wg, bg, Wu, bu, convg_w, convg_b, convu_w, convu_b, Wd, bd):
    in_maps = _prep_inputs(x, Wg, bg, Wu, bu, convg_w, convg_b,
                           convu_w, convu_b, Wd)
    res = run_on_cores(in_maps)
    acc = np.zeros((D, B * L), np.float32)
    for r in res.results:
        acc += np.asarray(r["yT"], np.float32)
    acc += np.asarray(bd, np.float32)[:, None]
    return np.ascontiguousarray(acc.T.reshape(B, L, D)).astype(np.float32)


# revision 14
# speedup vs baseline: 1.3775x; 1.0757x over previous
"""ConvSwiGLU Trainium2 kernel: tensor-parallel over d_ff across 8 NeuronCores.

v4 design. Profile history: v2 408us (ACT 90%/DVE 93%); v3 537us
(dual-scalar tensor_scalar loses fast DVE modes; merged 3D tensor_tensor
pays ~150-220cyc/row); v3.1 426us (PE idled 78us on evac backlogs + head +
tail); v3.2 420us (chunk-granular conv doubled DVE op count; microbench
shows DVE per-op fixed overhead ~92ns TT / ~160ns TS regardless of width,
so DVE wants FEW, WIDE ops).

  - All matmuls bf16 (fp8 DoubleRow is 2x but needs 3-term error
    compensation at this tolerance -> 1.5x bf16 cost; dead end).
  - Center-tap elimination: all taps of each side are scaled by 1/w2 on
    the host so the center tap is exactly 1.0 and the add tree reads the
    h slab DIRECTLY for it (no center premult op at all). The gate side
    is rescaled for free inside Silu's affine (scale=w2g, bias=tb2g); the
    up side's w2u is folded into Wd's rows on the host, and its bias
    tb2u/w2u rides the j3u ACT tap. Scaling by a constant keeps bf16
    RELATIVE precision, so numerics are unchanged (w2 clamped at 1e-6).
  - Conv at chunk-PAIR width (1024): premult tiles [128, GRP, 1028];
    DVE: j0/j4 @4x + j1 @2x_2p (24 TS/pair) + per-group add tree
    (32 TT/pair) + swiglu mult (4 TT/pair) ~= 18.2us/chunk.
    ACT: j3 both sides (8 taps/pair, shift-insensitive, up one carries
    the folded bias) + per-group Silu + 16 pure-copy [128,2,512] 2-bank
    psum evacuations ~= 16.3us/chunk. PE 20.4us/chunk paces.
  - Slab halo columns hold -b so pad taps contribute exactly 0.
  - produce/down PE chains emitted half-interleaved (pc1 pc2 dc1 dc2 pc3
    pc4 dc3 dc4) so a produce's 3rd chain never waits on an evac queued
    behind a conv tap batch (ps_main/ps_dn are only 2 tiles deep).
  - Pipeline: slot i = produce(i) halves + down(i-3) halves + conv piece
    (pair p = chunks 2p,2p+1: premults+taps+gate adds at slot 2p+2, up
    adds+silu+mult at slot 2p+3). x chunk DMA split 4 ways for the head.
  - Down matmul: psum[m,t] = sum_f Wd'[f,m] hact_s[f,t]; per-core partial
    yT summed on the host (bf16 partials, f32 host accumulate).
"""

import os
import sys
from contextlib import ExitStack

import ml_dtypes
import numpy as np

for _p in ("/root/.axon_site/_ro/trn_rl_repo", "/opt/trn_rl_repo"):
    if os.path.isdir(_p) and _p not in sys.path:
        sys.path.append(_p)

import concourse.bass as bass
import concourse.tile as tile
from concourse import bacc, mybir
from concourse.bass_utils import run_bass_kernel_spmd

F32 = mybir.dt.float32
BF16 = mybir.dt.bfloat16
AF = mybir.ActivationFunctionType
ALU = mybir.AluOpType

B, L, D = 4, 2048, 1024
F = 4096
NCORES = 8
FS = F // NCORES          # 512 d_ff channels per core
KSUB = D // 128           # 8 contraction subtiles for gate/up
GRP = FS // 128           # 4 channel groups per core
MSUB = D // 128           # 8 output row subtiles for down matmul
T = 512                   # token chunk (psum bank limit for f32)
CPS = L // T              # 4 chunks per sequence
NCH = (B * L) // T        # 16 chunks
NSEQ = B                  # 4 sequences
K = 5                     # conv taps
SLAB_W = L + 6            # 2 halo + 2048 tokens + 4 halo/pad
PW = 1024                 # conv pair width (2 chunks)
PRW = PW + 4              # premult tile width

_cache = {}


def _build_program():
    nc = bacc.Bacc("TRN2", target_bir_lowering=False, debug=False,
                   enable_asserts=False, num_devices=NCORES)

    xTc = nc.dram_tensor("xTc", [NCH, 128, KSUB, T], BF16, kind="ExternalInput").ap()
    wg = nc.dram_tensor("wgS", [128, KSUB, FS], BF16, kind="ExternalInput").ap()
    wu = nc.dram_tensor("wuS", [128, KSUB, FS], BF16, kind="ExternalInput").ap()
    wd = nc.dram_tensor("wdS", [128, GRP, D], BF16, kind="ExternalInput").ap()
    tsc = nc.dram_tensor("tscS", [128, 2, K, GRP], F32, kind="ExternalInput").ap()
    tb2 = nc.dram_tensor("tb2S", [128, 2, GRP], F32, kind="ExternalInput").ap()
    sw2 = nc.dram_tensor("sw2S", [128, GRP], F32, kind="ExternalInput").ap()
    nbh = nc.dram_tensor("nbhS", [128, 2, GRP, 6], BF16, kind="ExternalInput").ap()
    yT = nc.dram_tensor("yT", [D, B * L], BF16, kind="ExternalOutput").ap()

    with tile.TileContext(nc) as tc, ExitStack() as ctx:
        consts = ctx.enter_context(tc.tile_pool(name="consts", bufs=1))
        xpool = ctx.enter_context(tc.tile_pool(name="x", bufs=3))
        pp1 = ctx.enter_context(tc.tile_pool(name="pp1", bufs=1))
        ab1 = ctx.enter_context(tc.tile_pool(name="ab1", bufs=2))
        ab2 = ctx.enter_context(tc.tile_pool(name="ab2", bufs=2))
        hapool = ctx.enter_context(tc.tile_pool(name="ha", bufs=2))
        outpool = ctx.enter_context(tc.tile_pool(name="out", bufs=2))
        ps_main = ctx.enter_context(tc.tile_pool(name="psm", bufs=2, space="PSUM"))
        ps_dn = ctx.enter_context(tc.tile_pool(name="psd", bufs=2, space="PSUM"))

        # resident weights / constants
        wg_sb = consts.tile([128, KSUB, FS], BF16)
        wu_sb = consts.tile([128, KSUB, FS], BF16)
        wd_sb = consts.tile([128, GRP, D], BF16)
        tsc_sb = consts.tile([128, 2, K, GRP], F32)
        tb2_sb = consts.tile([128, 2, GRP], F32)
        sw2_sb = consts.tile([128, GRP], F32)
        # const loads on the Activation DMA queue (x/out use the SP queue);
        # wg first 2 ksubs first so chunk 0 matmuls can start immediately
        nc.scalar.dma_start(wg_sb[:, 0:2], wg[:, 0:2])
        nc.scalar.dma_start(wg_sb[:, 2:8], wg[:, 2:8])
        nc.scalar.dma_start(wu_sb[:], wu)
        nc.scalar.dma_start(wd_sb[:], wd)
        nc.scalar.dma_start(tsc_sb[:], tsc)
        nc.scalar.dma_start(tb2_sb[:], tb2)
        nc.scalar.dma_start(sw2_sb[:], sw2)

        # one shared h slab (sides x groups x padded seq); halo columns hold
        # -b so pure-mult premults contribute exactly 0 at pad positions.
        # Evacuations only ever touch the interior [2, 2+L).
        slab = consts.tile([128, 2, GRP, SLAB_W], BF16)
        nc.gpsimd.dma_start(slab[:, :, :, 0:2], nbh[:, :, :, 0:2])
        nc.gpsimd.dma_start(slab[:, :, :, 2 + L:SLAB_W], nbh[:, :, :, 2:6])

        ha_tiles = {}

        def produce_half(i, part):
            """gate (part 0) or up (part 1) matmuls for chunk i."""
            c = i % CPS
            if part == 0:
                xt = xpool.tile([128, KSUB, T], BF16, tag="xt")
                for k0 in range(0, KSUB, 2):
                    nc.sync.dma_start(xt[:, k0:k0 + 2], xTc[i, :, k0:k0 + 2])
                produce_half.xt = xt
            xt = produce_half.xt
            sd, w_sb = ((0, wg_sb), (1, wu_sb))[part]
            for gp in range(2):
                ps = ps_main.tile([128, 2, T], F32, tag="mm1")
                for half in range(2):
                    g = gp * 2 + half
                    for ks in range(KSUB):
                        nc.tensor.matmul(
                            ps[:, half, :],
                            w_sb[:, ks, g * 128:(g + 1) * 128],
                            xt[:, ks, :],
                            start=(ks == 0), stop=(ks == KSUB - 1))
                nc.scalar.activation(
                    slab[:, sd, 2 * gp:2 * gp + 2, 2 + c * T:2 + (c + 1) * T],
                    ps[:], AF.Identity)

        def conv_a(p):
            """pair p (chunks 2p,2p+1): premults + taps + gate add tree."""
            t0 = (p % 2) * PW
            P = {}
            for sd in (0, 1):
                for j in (0, 1, 3, 4):
                    pj = pp1.tile([128, GRP, PRW], BF16, tag=f"p{sd}_{j}")
                    P[sd, j] = pj
                    dlt = j % 2
                    for g in range(GRP):
                        src = slab[:, sd, g, t0 + dlt:t0 + dlt + PRW]
                        sc = tsc_sb[:, sd, j, g:g + 1]
                        if j == 3:
                            # ACT: 1x, shift-insensitive; up side carries the
                            # folded (scaled) bias
                            bias = tb2_sb[:, 1, g:g + 1] if sd else 0.0
                            nc.scalar.activation(pj[:, g, :], src, AF.Identity,
                                                 bias=bias, scale=sc)
                        else:
                            # DVE: j0/j4 at 4x (aligned), j1 at 2x_2p
                            nc.vector.tensor_scalar(pj[:, g, :], src, sc, None,
                                                    ALU.mult)
            conv_a.P = P
            _tree(P, 0, t0)

        def _tree(P, sd, t0):
            """per-group add tree; center tap reads the slab directly
            (coefficient 1.0 after the 1/w2 host scaling). tc aliases into
            p0's row and r into p4's row (both already consumed by ta)."""
            p0, p1, p3, p4 = (P[sd, j] for j in (0, 1, 3, 4))
            for g in range(GRP):
                ta = ab1.tile([128, PW], BF16, tag="ta")
                tb = ab1.tile([128, PW], BF16, tag="tb")
                nc.vector.tensor_tensor(ta[:], p0[:, g, 0:PW],
                                        p4[:, g, 4:4 + PW], ALU.add)
                nc.vector.tensor_tensor(tb[:], p1[:, g, 0:PW],
                                        p3[:, g, 2:2 + PW], ALU.add)
                nc.vector.tensor_tensor(p0[:, g, 0:PW], ta[:], tb[:], ALU.add)
                nc.vector.tensor_tensor(p4[:, g, 0:PW], p0[:, g, 0:PW],
                                        slab[:, sd, g, t0 + 2:t0 + 2 + PW],
                                        ALU.add)

        def conv_b(p):
            """pair p: up add tree + silu (rescales gate) + swiglu mult."""
            t0 = (p % 2) * PW
            P = conv_a.P
            _tree(P, 1, t0)
            ha_t = hapool.tile([128, GRP, PW], BF16, tag="ha")
            for g in range(GRP):
                gact = ab2.tile([128, PW], BF16, tag="gact")
                nc.scalar.activation(gact[:], P[0, 4][:, g, 0:PW], AF.Silu,
                                     bias=tb2_sb[:, 0, g:g + 1],
                                     scale=sw2_sb[:, g:g + 1])
                nc.vector.tensor_tensor(ha_t[:, g, :], gact[:],
                                        P[1, 4][:, g, 0:PW], ALU.mult)
            ha_tiles[p] = ha_t

        yTr = yT.rearrange("(ms p) t -> p ms t", p=128)

        def down_half(i, part):
            """down matmul pair-chains 2*part..2*part+1 for chunk i."""
            ha_t = ha_tiles[i // 2]
            off = (i % 2) * T
            if part == 0:
                down_half.out = outpool.tile([128, MSUB, T], BF16, tag="out")
            out_sb = down_half.out
            for mp in (2 * part, 2 * part + 1):
                dps = ps_dn.tile([128, 2, T], F32, tag="dn")
                for half in range(2):
                    ms = mp * 2 + half
                    for g in range(GRP):
                        nc.tensor.matmul(
                            dps[:, half, :],
                            wd_sb[:, g, ms * 128:(ms + 1) * 128],
                            ha_t[:, g, off:off + T],
                            start=(g == 0), stop=(g == GRP - 1))
                nc.scalar.activation(out_sb[:, 2 * mp:2 * mp + 2, :], dps[:],
                                     AF.Identity)
            if part == 1:
                nc.gpsimd.dma_start(yTr[:, :, i * T:(i + 1) * T], out_sb[:])

        # slot i: produce(i) + down(i-3), chains half-interleaved; conv
        # pieces: pair p gets conv_a at slot 2p+2, conv_b at slot 2p+3
        # (emitted before down(2p), which consumes its ha)
        for i in range(NCH):
            d = i - 3
            if i % 2 == 0:
                produce_half(i, 0)
                if d >= 0:
                    down_half(d, 0)
                produce_half(i, 1)
                if d >= 0:
                    down_half(d, 1)
                if i >= 2:
                    conv_a(i // 2 - 1)
            else:
                produce_half(i, 0)
                if i >= 3:
                    conv_b(i // 2 - 1)
                if d >= 0:
                    down_half(d, 0)
                produce_half(i, 1)
                if d >= 0:
                    down_half(d, 1)
        conv_a(NCH // 2 - 1)
        conv_b(NCH // 2 - 1)
        for d in (NCH - 3, NCH - 2, NCH - 1):
            down_half(d, 0)
            down_half(d, 1)

    nc.compile()
    return nc


def _prep_inputs(x, Wg, bgv, Wu, buv, convg_w, convg_b, convu_w, convu_b, Wd):
    """Host-side shard/layout. Returns list of per-core in_maps."""
    bf16 = ml_dtypes.bfloat16
    x = np.ascontiguousarray(x, np.float32)
    # [B, L, D] -> [B, KSUB, 128, L] -> chunks [NCH, 128, KSUB, T]
    xt = x.transpose(0, 2, 1).reshape(B, KSUB, 128, L)
    xTc = np.stack([
        xt[i // CPS, :, :, (i % CPS) * T:(i % CPS + 1) * T].transpose(1, 0, 2)
        for i in range(NCH)
    ]).astype(bf16)

    def colsplit(w, c):      # [D, F] -> per-core [128, KSUB, FS]
        s = np.asarray(w, np.float32)[:, c * FS:(c + 1) * FS]
        return np.ascontiguousarray(
            s.reshape(KSUB, 128, FS).transpose(1, 0, 2)).astype(bf16)

    def vecsplit(v, c):      # [F] -> [128, GRP] f32
        return np.ascontiguousarray(
            np.asarray(v, np.float32)[c * FS:(c + 1) * FS].reshape(GRP, 128).T)

    def tapsplit(w, c):      # [F, 1, K] -> [128, GRP, K] f32
        return np.ascontiguousarray(
            np.asarray(w, np.float32)[c * FS:(c + 1) * FS, 0, :]
            .reshape(GRP, 128, K).transpose(1, 0, 2))

    def clamp(w):            # keep sign, floor magnitude at 1e-6
        return np.where(np.abs(w) < 1e-6, np.where(w < 0, -1e-6, 1e-6), w)

    in_maps = []
    for c in range(NCORES):
        # per-side scaling: taps / w2 (center tap == 1.0, read directly);
        # gate rescaled inside silu (scale=w2g, bias=tb2g); up w2u folded
        # into Wd rows, up bias (scaled) rides the j3u tap
        tscS = np.zeros((128, 2, K, GRP), np.float32)
        tb2S = np.zeros((128, 2, GRP), np.float32)
        nbhS = np.zeros((128, 2, GRP, 6), np.float32)
        w2 = {}
        for sd, (cw, bv, cb) in enumerate(((convg_w, bgv, convg_b),
                                           (convu_w, buv, convu_b))):
            taps = tapsplit(cw, c)            # [128, GRP, K]
            bias = vecsplit(bv, c)            # [128, GRP]
            w2[sd] = clamp(taps[:, :, 2])
            tscS[:, sd] = (taps / w2[sd][:, :, None]).transpose(0, 2, 1)
            tb2S[:, sd] = bias * taps.sum(axis=2) + vecsplit(cb, c)
            nbhS[:, sd] = -bias[:, :, None]
        tb2S[:, 1] /= w2[1]                   # up bias lives in scaled domain
        sw2S = w2[0]                          # silu scale rescales the gate
        wdS = np.asarray(Wd, np.float32)[c * FS:(c + 1) * FS, :]
        wdS = wdS * w2[1].T.reshape(FS, 1)    # fold w2u into Wd rows
        in_maps.append({
            "xTc": xTc,
            "wgS": colsplit(Wg, c),
            "wuS": colsplit(Wu, c),
            "wdS": np.ascontiguousarray(
                wdS.reshape(GRP, 128, D).transpose(1, 0, 2)).astype(bf16),
            "tscS": tscS,
            "tb2S": tb2S,
            "sw2S": sw2S,
            "nbhS": nbhS.astype(bf16),
        })
    return in_maps


def run_on_cores(in_maps, **kwargs):
    if "nc" not in _cache:
        _cache["nc"] = _build_program()
    return run_bass_kernel_spmd(_cache["nc"], in_maps,
                                core_ids=list(range(NCORES)), **kwargs)


def kernel(x, Wg, bg, Wu, bu, convg_w, convg_b, convu_w, convu_b, Wd, bd):
    in_maps = _prep_inputs(x, Wg, bg, Wu, bu, convg_w, convg_b,
                           convu_w, convu_b, Wd)
    res = run_on_cores(in_maps)
    acc = np.zeros((D, B * L), np.float32)
    for r in res.results:
        acc += np.asarray(r["yT"], np.float32)
    acc += np.asarray(bd, np.float32)[:, None]
    return np.ascontiguousarray(acc.T.reshape(B, L, D)).astype(np.float32)
